# revision 1
# baseline (speedup 1.0000x reference)
"""Trainium2 Bass kernel for CosineGraphAttentionLayer.

reference:
    cos = beta * (xi @ xj.T) / (|xi| |xj| + eps)
    P   = softmax(cos + (1-adj) * -1e9, axis=1)
    out = P @ xj

Sharding: xi/adj row-sharded across 8 cores, xj/beta replicated (no collectives).

Per-core kernel design (scores kept in TRANSPOSED [j, i] layout throughout):
  - fold beta/|xi| into xi rows and 1/|xj| into xj rows (eps dropped: its
    relative effect is ~eps/D ~ 4e-10, far below fp32 rounding)
  - MM1: ST[j, i] = xj_s @ xi_s.T  via PE (lhsT = xj_s.T, rhs = xi_s.T,
    both made on-chip with PE-transposes)
  - E = exp(ST) on ACT straight out of PSUM (|ST| <= beta <= 1, so no
    row-max subtraction is needed for stability)
  - mask: Pu = E * adjT.  adj is streamed in as fp16 via gpsimd cast-DMA
    (int32 -> fp16) and transposed on-chip with the DMA xbar transpose
    (2-byte dtype, per-128-block 3D-out form)
  - MM2: out[i, 0:256] += Pu_tile.T @ [xj | 1] accumulated over all j in
    PSUM; column 256 accumulates the softmax denominator
  - normalize rows by 1/den on DVE, DMA out
"""
import sys

sys.path.insert(0, "/opt/trn_rl_repo")

import numpy as np

import concourse.bass as bass
import concourse.bacc as bacc
import concourse.tile as tile
from concourse import mybir, masks
from concourse.bass_utils import run_bass_kernel_spmd

F32 = mybir.dt.float32
F16 = mybir.dt.float16
I32 = mybir.dt.int32

N_CORES = 8


def build_nc(NI=1024, M=8192, D=256):
    """Build the per-core bass program. NI = rows per core, M = columns (j), D = feature dim."""
    assert NI % 256 == 0 and M % 1024 == 0 and D == 256
    NIB = NI // 128          # i-blocks per core
    NHALF = 2                # i halves (PSUM capacity: 4 out banks per half)
    IBH = NIB // NHALF       # i-blocks per half
    IW = IBH * 128           # i width per half
    NJB = M // 128           # j blocks
    NJC = M // 1024          # j chunks (adj staging granularity)
    DH = D // 128            # d halves

    nc = bacc.Bacc("TRN2", target_bir_lowering=False, debug=False)
    xi = nc.declare_dram_parameter("xi", [NI, D], F32, isOutput=False)
    xj = nc.declare_dram_parameter("xj", [M, D], F32, isOutput=False)
    adj = nc.declare_dram_parameter("adj", [NI, M], I32, isOutput=False)
    beta = nc.declare_dram_parameter("beta", [1], F32, isOutput=False)
    out = nc.declare_dram_parameter("out", [NI, D], F32, isOutput=True)

    with tile.TileContext(nc) as tc:
        with (
            tc.tile_pool(name="big", bufs=1) as big,
            tc.tile_pool(name="prep", bufs=3) as prep,
            tc.tile_pool(name="adjp", bufs=2) as adjp,
            tc.tile_pool(name="work", bufs=3) as work,
            tc.tile_pool(name="outp", bufs=4) as outp,
            tc.tile_pool(name="ps_s", bufs=2, space="PSUM") as ps_s,
            tc.tile_pool(name="ps_o", bufs=IBH, space="PSUM") as ps_o,
            tc.tile_pool(name="ps_t", bufs=2, space="PSUM") as ps_t,
        ):
            # ---------------- static tiles ----------------
            # xj with a ones column appended (rhs of MM2), grouped 8 j-blocks per tile
            xj_aug = [big.tile([128, 8, D + 1], F32, name=f"xj_aug{g}", tag=f"xj_aug{g}")
                      for g in range(NJB // 8)]
            # transposed scaled xj / xi, one tile per d-half
            xj_sT = [big.tile([128, M], F32, name=f"xj_sT{dh}", tag=f"xj_sT{dh}") for dh in range(DH)]
            xi_sT = [big.tile([128, NI], F32, name=f"xi_sT{dh}", tag=f"xi_sT{dh}") for dh in range(DH)]
            ident = big.tile([128, 128], F32)
            beta_sb = big.tile([128, 1], F32)
            ssq_j = big.tile([128, NJB], F32)
            ssq_i = big.tile([128, NIB], F32)
            rj = big.tile([128, NJB], F32)
            ri = big.tile([128, NIB], F32)

            masks.make_identity(nc, ident[:, :])
            nc.scalar.dma_start(
                out=beta_sb[:, :],
                in_=bass.AP(tensor=beta, offset=0, ap=[[0, 128], [1, 1]]),
            )

            # ---------------- prep: load xj, norms, scale, transpose ----------------
            # xj DRAM [M, D] -> [128, jb, D] partition-major within 128-row blocks
            xj_r = xj[:, :].rearrange("(jb p) d -> p jb d", p=128)
            xi_r = xi[:, :].rearrange("(ib p) d -> p ib d", p=128)
            for g in range(NJB // 8):
                nc.scalar.dma_start(
                    out=xj_aug[g][:, :, 0:D], in_=xj_r[:, 8 * g:8 * (g + 1), :]
                )
                nc.vector.memset(xj_aug[g][:, :, D:D + 1], 1.0)

            xi_all = big.tile([128, NIB, D], F32)
            nc.scalar.dma_start(out=xi_all[:, :, :], in_=xi_r[:, :, :])

            # row sums of squares via ACT Square + accum_out
            for jb in range(NJB):
                sq = prep.tile([128, D], F32, tag="sq")
                nc.scalar.activation(
                    out=sq[:, :], in_=xj_aug[jb // 8][:, jb % 8, 0:D],
                    func=mybir.ActivationFunctionType.Square,
                    accum_out=ssq_j[:, jb:jb + 1],
                )
            for ib in range(NIB):
                sq = prep.tile([128, D], F32, tag="sq")
                nc.scalar.activation(
                    out=sq[:, :], in_=xi_all[:, ib, :],
                    func=mybir.ActivationFunctionType.Square,
                    accum_out=ssq_i[:, ib:ib + 1],
                )
            # rj = 1/sqrt(ssq_j); ri = beta/sqrt(ssq_i)
            nc.scalar.activation(out=ssq_j[:, :], in_=ssq_j[:, :],
                                 func=mybir.ActivationFunctionType.Sqrt)
            nc.vector.reciprocal(out=rj[:, :], in_=ssq_j[:, :])
            nc.scalar.activation(out=ssq_i[:, :], in_=ssq_i[:, :],
                                 func=mybir.ActivationFunctionType.Sqrt)
            nc.vector.reciprocal(out=ri[:, :], in_=ssq_i[:, :])
            nc.vector.tensor_scalar(out=ri[:, :], in0=ri[:, :],
                                    scalar1=beta_sb[:, 0:1], scalar2=None,
                                    op0=mybir.AluOpType.mult)

            # scale rows then PE-transpose into xj_sT / xi_sT
            for jb in range(NJB):
                t = prep.tile([128, D], F32, tag="xjs")
                nc.vector.tensor_scalar(out=t[:, :], in0=xj_aug[jb // 8][:, jb % 8, 0:D],
                                        scalar1=rj[:, jb:jb + 1], scalar2=None,
                                        op0=mybir.AluOpType.mult)
                for dh in range(DH):
                    tp = ps_t.tile([128, 128], F32, tag="tp")
                    nc.tensor.matmul(tp[:, :], t[:, 128 * dh:128 * (dh + 1)],
                                     ident[:, :], is_transpose=True)
                    nc.vector.tensor_copy(
                        xj_sT[dh][:, 128 * jb:128 * (jb + 1)], tp[:, :])
            for ib in range(NIB):
                t = prep.tile([128, D], F32, tag="xis")
                nc.vector.tensor_scalar(out=t[:, :], in0=xi_all[:, ib, :],
                                        scalar1=ri[:, ib:ib + 1], scalar2=None,
                                        op0=mybir.AluOpType.mult)
                for dh in range(DH):
                    tp = ps_t.tile([128, 128], F32, tag="tp")
                    nc.tensor.matmul(tp[:, :], t[:, 128 * dh:128 * (dh + 1)],
                                     ident[:, :], is_transpose=True)
                    nc.vector.tensor_copy(
                        xi_sT[dh][:, 128 * ib:128 * (ib + 1)], tp[:, :])

            # ---------------- main loop ----------------
            adj16_r = adj[:, :].rearrange("i (jc q) -> i jc q", q=1024)  # int32 view, 1024-col chunks
            for h in range(NHALF):
                ps_out = [ps_o.tile([128, D + 1], F32, name=f"ps_out_{h}_{bb}", tag="ps_out")
                          for bb in range(IBH)]
                for jc in range(NJC):
                    # stage adj chunk: int32 load + gpsimd cast + xbar transpose
                    adjT = adjp.tile([128, IBH, 8, 128], F16, tag="adjT")
                    for b in range(IBH):
                        ib = h * IBH + b
                        a32 = work.tile([128, 1024], I32, tag="a32")
                        nc.scalar.dma_start(
                            out=a32[:, :],
                            in_=adj16_r[128 * ib:128 * (ib + 1), jc, :],
                        )
                        a16 = work.tile([128, 1024], F16, tag="a16")
                        nc.gpsimd.tensor_copy(a16[:, :], a32[:, :])
                        nc.sync.dma_start_transpose(
                            out=adjT[:, b, :, :], in_=a16[:, :])
                    for q in range(8):
                        jb = 8 * jc + q
                        # MM1: ST[j=128, i=IW]
                        st = ps_s.tile([128, IW], F32, tag="st")
                        for dh in range(DH):
                            nc.tensor.matmul(
                                st[:, :],
                                xj_sT[dh][:, 128 * jb:128 * (jb + 1)],
                                xi_sT[dh][:, IW * h:IW * (h + 1)],
                                start=(dh == 0), stop=(dh == DH - 1),
                            )
                        e = work.tile([128, IW], F32, tag="e")
                        nc.scalar.activation(
                            out=e[:, :], in_=st[:, :],
                            func=mybir.ActivationFunctionType.Exp)
                        pu = work.tile([128, IW], F32, tag="pu")
                        nc.vector.tensor_tensor(
                            out=pu[:, :].rearrange("j (b i) -> j b i", b=IBH),
                            in0=e[:, :].rearrange("j (b i) -> j b i", b=IBH),
                            in1=adjT[:, :, q, :],
                            op=mybir.AluOpType.mult,
                        )
                        # MM2: out[i, :] += Pu_tile.T @ xj_aug
                        for b in range(IBH):
                            nc.tensor.matmul(
                                ps_out[b][:, :],
                                pu[:, 128 * b:128 * (b + 1)],
                                xj_aug[jb // 8][:, jb % 8, :],
                                start=(jb == 0), stop=(jb == NJB - 1),
                            )
                # normalize + store
                for b in range(IBH):
                    ib = h * IBH + b
                    rden = outp.tile([128, 1], F32, tag="rden")
                    nc.vector.reciprocal(out=rden[:, :], in_=ps_out[b][:, D:D + 1])
                    of = outp.tile([128, D], F32, tag="of")
                    nc.vector.tensor_scalar(
                        out=of[:, :], in0=ps_out[b][:, 0:D],
                        scalar1=rden[:, 0:1], scalar2=None,
                        op0=mybir.AluOpType.mult)
                    nc.scalar.dma_start(
                        out=out[128 * ib:128 * (ib + 1), :], in_=of[:, :])

    nc.finalize()
    return nc


_NC_CACHE = {}


def _get_nc(NI, M, D):
    key = (NI, M, D)
    if key not in _NC_CACHE:
        _NC_CACHE[key] = build_nc(NI, M, D)
    return _NC_CACHE[key]


def kernel(xi, xj, adj, beta):
    xi = np.ascontiguousarray(np.asarray(xi, dtype=np.float32))
    xj = np.ascontiguousarray(np.asarray(xj, dtype=np.float32))
    adj = np.ascontiguousarray(np.asarray(adj, dtype=np.int32))
    beta = np.ascontiguousarray(np.asarray(beta, dtype=np.float32))
    N, D = xi.shape
    M = xj.shape[0]
    NI = N // N_CORES
    nc = _get_nc(NI, M, D)
    in_maps = [
        {
            "xi": xi[k * NI:(k + 1) * NI],
            "xj": xj,
            "adj": adj[k * NI:(k + 1) * NI],
            "beta": beta,
        }
        for k in range(N_CORES)
    ]
    res = run_bass_kernel_spmd(nc, in_maps, list(range(N_CORES)))
    return np.concatenate([res.results[k]["out"] for k in range(N_CORES)], axis=0)



# revision 3
# speedup vs baseline: 2.9900x; 2.9900x over previous
"""Trainium2 Bass kernel for CosineGraphAttentionLayer.

reference:
    cos = beta * (xi @ xj.T) / (|xi| |xj| + eps)
    P   = softmax(cos + (1-adj) * -1e9, axis=1)
    out = P @ xj

Sharding: xi/adj row-sharded across 8 cores; xj sharded and AllGathered
on-device (NeuronLink), beta folded into xi host-side.

The dominant cost of this problem under the axon-tunneled harness is
per-execution input shipping (~10 GB/s), so inputs are shipped compact:
  - adj as bit-packed int16 words (32x smaller than int32)
  - xi pre-normalized (beta/|xi| folded in), transposed, f16
  - xj as f16 shards, AllGathered across the 8 cores on-device
  - 1/|xj| as a small f32 vector (folds into the exp via ACT's scale)
  - out returned as f16, cast to f32 on host

Per-core kernel (scores in TRANSPOSED [j, i] layout, all matmuls f16):
  - AllGather xj shards -> full xj f16 in DRAM; load as [128, jb, 257]
    with a ones column appended (MM2 rhs; col 256 accumulates the
    softmax denominator)
  - PE-transpose xj blocks -> xjT [d, j] tiles (MM1 lhsT)
  - MM1: ST[j,i] = xjT.T @ xi_sT, f16, N=512
  - exp on ACT straight out of PSUM with scale=1/|xj_j| per partition
    (|arg| <= beta <= 1, so no row-max needed for stability)
  - adj mask: packed bits unpacked on DVE via (w >> b) & 1 -> {0,1} i16,
    applied to E via an int16 multiply of the f16 bit patterns
    (x*1 = x, x*0 = 0 bitwise-exact)
  - MM2: out[i, 0:257] += Pu.T @ [xj | 1] accumulated over all j in PSUM
  - normalize rows by 1/den on DVE, DMA out as f16
"""
import sys

sys.path.insert(0, "/opt/trn_rl_repo")

import numpy as np

import concourse.bass as bass
import concourse.bacc as bacc
import concourse.tile as tile
from concourse import mybir, masks
from concourse.bass_utils import run_bass_kernel_spmd

F32 = mybir.dt.float32
F16 = mybir.dt.float16
I16 = mybir.dt.int16

N_CORES = 8


def build_nc(NI=1024, M=8192, D=256):
    """Per-core program. NI = i-rows per core, M = j-columns, D = features."""
    assert NI == 1024 and M % 1024 == 0 and D == 256
    NIB = NI // 128          # i-blocks per core (8)
    NHALF = 2                # i halves
    IBH = NIB // NHALF       # i-blocks per half (4)
    IW = NI // NHALF         # i width per half (512)
    NJB = M // 128           # j-blocks (64)
    NG = NJB // 8            # groups of 8 j-blocks (8)
    DH = D // 128            # d halves (2)
    W = NI // 16             # packed words per j-row (64)
    SH = M // N_CORES        # xj shard rows (1024)

    nc = bacc.Bacc("TRN2", target_bir_lowering=False, debug=False)
    xi_t = nc.declare_dram_parameter("xi_t", [D, NI], F16, isOutput=False)
    xjs = nc.declare_dram_parameter("xjs", [SH, D], F16, isOutput=False)
    adjp = nc.declare_dram_parameter("adjp", [128, NJB * W], I16, isOutput=False)
    sj = nc.declare_dram_parameter("sj", [128, NJB], F32, isOutput=False)
    out = nc.declare_dram_parameter("out", [NI, D], F16, isOutput=True)

    xj_bounce = nc.dram_tensor("xj_bounce", [SH, D], F16, kind="Internal")
    xj_full = nc.dram_tensor("xj_full", [M, D], F16, kind="Internal",
                             addr_space="Shared")

    with tile.TileContext(nc) as tc:
        with (
            tc.tile_pool(name="big", bufs=1) as big,
            tc.tile_pool(name="mpool", bufs=2) as mpool,
            tc.tile_pool(name="epool", bufs=2) as epool,
            tc.tile_pool(name="ppool", bufs=2) as ppool,
            tc.tile_pool(name="outp", bufs=4) as outp,
            tc.tile_pool(name="ps_a", space="PSUM", bufs=2) as ps_a,
            tc.tile_pool(name="ps_o", space="PSUM", bufs=IBH) as ps_o,
        ):
            # ---------------- static tiles ----------------
            ident = big.tile([128, 128], F16)
            xi_sT = big.tile([128, DH, NI], F16)
            sj_sb = big.tile([128, NJB], F32)
            pk = big.tile([128, NJB, W], I16)
            xj_aug = big.tile([128, NJB, D + 1], F16)
            xjT = [big.tile([128, M], F16, name=f"xjT{dh}", tag=f"xjT{dh}")
                   for dh in range(DH)]

            masks.make_identity(nc, ident[:, :])

            # ---------------- prep ----------------
            nc.sync.dma_start(out=xj_bounce[:, :], in_=xjs[:, :])
            nc.gpsimd.collective_compute(
                "AllGather", mybir.AluOpType.bypass,
                replica_groups=[list(range(N_CORES))],
                ins=[xj_bounce[:, :]], outs=[xj_full[:, :]],
            )
            nc.scalar.dma_start(
                out=xi_sT[:, :, :],
                in_=xi_t[:, :].rearrange("(dh p) i -> p dh i", p=128))
            nc.scalar.dma_start(out=sj_sb[:, :], in_=sj[:, :])
            nc.scalar.dma_start(
                out=pk[:, :, :],
                in_=adjp[:, :].rearrange("p (jb w) -> p jb w", w=W))
            nc.vector.memset(xj_aug[:, :, D:D + 1], 1.0)
            nc.scalar.dma_start(
                out=xj_aug[:, :, 0:D],
                in_=xj_full[:, :].rearrange("(jb p) d -> p jb d", p=128))

            # PE-transpose xj -> xjT [d, j], 8 j-blocks per PSUM bank (f16)
            for dh in range(DH):
                for g8 in range(NJB // 8):
                    tp = ps_a.tile([128, 8, 128], F16, tag="tp")
                    for q in range(8):
                        jb = 8 * g8 + q
                        nc.tensor.matmul(
                            tp[:, q, :], xj_aug[:, jb, 128 * dh:128 * (dh + 1)],
                            ident[:, :], is_transpose=True)
                    nc.vector.tensor_copy(
                        xjT[dh][:, 1024 * g8:1024 * (g8 + 1)], tp[:, :, :])

            # ---------------- main loop ----------------
            for h in range(NHALF):
                ps_out = [ps_o.tile([128, D + 1], F32, name=f"ps_out_{h}_{b}",
                                    tag="ps_out") for b in range(IBH)]
                for g in range(NG):
                    # unpack 8 j-blocks' mask bits for this i-half -> {0,1} i16
                    mask = mpool.tile([128, 8, IW], I16, tag="mask")
                    for bb in range(8):
                        nc.vector.tensor_scalar(
                            out=mask[:, :, 64 * bb:64 * (bb + 1)],
                            in0=pk[:, 8 * g:8 * (g + 1), :],
                            scalar1=8 * h + bb, scalar2=1,
                            op0=mybir.AluOpType.logical_shift_right,
                            op1=mybir.AluOpType.bitwise_and,
                        )
                    e_ch = epool.tile([128, 8, IW], F16, tag="e")
                    for q in range(8):
                        jb = 8 * g + q
                        st = ps_a.tile([128, IW], F32, tag="st")
                        for dh in range(DH):
                            nc.tensor.matmul(
                                st[:, :],
                                xjT[dh][:, 128 * jb:128 * (jb + 1)],
                                xi_sT[:, dh, IW * h:IW * (h + 1)],
                                start=(dh == 0), stop=(dh == DH - 1),
                            )
                        nc.scalar.activation(
                            out=e_ch[:, q, :], in_=st[:, :],
                            func=mybir.ActivationFunctionType.Exp,
                            scale=sj_sb[:, jb:jb + 1])
                    # Pu = E * mask, as an i16 multiply of the f16 bit patterns
                    pu = ppool.tile([128, 8, IW], F16, tag="pu")
                    nc.vector.tensor_tensor(
                        out=pu[:, :, :].bitcast(I16),
                        in0=e_ch[:, :, :].bitcast(I16),
                        in1=mask[:, :, :], op=mybir.AluOpType.mult)
                    for q in range(8):
                        jb = 8 * g + q
                        for b in range(IBH):
                            nc.tensor.matmul(
                                ps_out[b][:, :],
                                pu[:, q, 128 * b:128 * (b + 1)],
                                xj_aug[:, jb, :],
                                start=(jb == 0), stop=(jb == NJB - 1),
                            )
                # normalize + store
                for b in range(IBH):
                    ib = h * IBH + b
                    rden = outp.tile([128, 1], F32, tag="rden")
                    nc.vector.reciprocal(out=rden[:, :], in_=ps_out[b][:, D:D + 1])
                    of = outp.tile([128, D], F16, tag="of")
                    nc.vector.tensor_scalar(
                        out=of[:, :], in0=ps_out[b][:, 0:D],
                        scalar1=rden[:, 0:1], scalar2=None,
                        op0=mybir.AluOpType.mult)
                    nc.scalar.dma_start(
                        out=out[128 * ib:128 * (ib + 1), :], in_=of[:, :])

    nc.finalize()
    return nc


_NC_CACHE = {}


def _get_nc(NI, M, D):
    key = (NI, M, D)
    if key not in _NC_CACHE:
        _NC_CACHE[key] = build_nc(NI, M, D)
    return _NC_CACHE[key]


def prepare_in_maps(xi, xj, adj, beta):
    """Host-side preprocessing: normalize/fold/pack the raw inputs into the
    compact per-core NEFF inputs."""
    xi = np.asarray(xi, dtype=np.float32)
    xj = np.asarray(xj, dtype=np.float32)
    adj = np.asarray(adj)
    beta = np.asarray(beta, dtype=np.float32)
    N, D = xi.shape
    M = xj.shape[0]
    NI = N // N_CORES
    SH = M // N_CORES
    NJB = M // 128
    W = NI // 16

    b = float(beta.reshape(-1)[0])
    xi_s = (xi * (b / np.linalg.norm(xi, axis=1, keepdims=True))).astype(np.float16)
    xj16 = xj.astype(np.float16)
    sj_all = (1.0 / np.linalg.norm(xj, axis=1)).astype(np.float32)
    # [128, NJB] with [p, jb] = 1/|xj_{128*jb+p}|
    sj_r = np.ascontiguousarray(sj_all.reshape(NJB, 128).T)

    # pack adj bits: word_c[j, w] bit b = adj[c*NI + 64*b + w, j]
    A = (adj != 0).reshape(N_CORES, 16, W, M)
    words = np.zeros((N_CORES, W, M), dtype=np.uint16)
    for bb in range(16):
        words |= A[:, bb, :, :].astype(np.uint16) << bb

    in_maps = []
    for c in range(N_CORES):
        # [128, NJB*W] with [p, jb*W + w] = word_c[128*jb+p, w]
        adjp_c = np.ascontiguousarray(
            words[c].T.reshape(NJB, 128, W).transpose(1, 0, 2).reshape(128, NJB * W)
        ).view(np.int16)
        in_maps.append({
            "xi_t": np.ascontiguousarray(xi_s[c * NI:(c + 1) * NI].T),
            "xjs": np.ascontiguousarray(xj16[c * SH:(c + 1) * SH]),
            "adjp": adjp_c,
            "sj": sj_r,
        })
    return in_maps


def kernel(xi, xj, adj, beta):
    N, D = np.asarray(xi).shape
    M = np.asarray(xj).shape[0]
    NI = N // N_CORES
    nc = _get_nc(NI, M, D)
    in_maps = prepare_in_maps(xi, xj, adj, beta)
    res = run_bass_kernel_spmd(nc, in_maps, list(range(N_CORES)))
    return np.concatenate(
        [res.results[k]["out"] for k in range(N_CORES)], axis=0
    ).astype(np.float32)


# revision 4
# speedup vs baseline: 4.6265x; 1.5473x over previous
"""Trainium2 Bass kernel for CosineGraphAttentionLayer.

reference:
    cos = beta * (xi @ xj.T) / (|xi| |xj| + eps)
    P   = softmax(cos + (1-adj) * -1e9, axis=1)
    out = P @ xj

Sharding: xi/adj row-sharded across 8 cores; xj sharded and AllGathered
on-device (NeuronLink), beta folded into xi host-side.

The dominant cost of this problem under the axon-tunneled harness is
per-execution dispatch overhead, which scales with the NUMBER of input
buffers (~2 ms each) plus shipped bytes (~12 GB/s). So all inputs are
packed host-side into a single compact int16 blob per core:
  - xi pre-normalized (beta/|xi| folded in), transposed, f16
  - xj as f16 shard (AllGathered across the 8 cores on-device)
  - 1/|xj| as f16 (folds into the exp via ACT's per-partition scale)
  - adj bit-packed into int16 words (32x smaller than int32)
and the output is returned as f16, cast to f32 on host.

Per-core kernel (scores in TRANSPOSED [j, i] layout, all matmuls f16):
  - AllGather xj shards -> full xj f16 in DRAM; load as [128, jb, 257]
    with a ones column appended (MM2 rhs; col 256 accumulates the
    softmax denominator)
  - PE-transpose xj blocks -> xjT [d, j] tiles (MM1 lhsT)
  - MM1: ST[j,i] = xjT.T @ xi_sT, f16, N=512
  - exp on ACT straight out of PSUM with scale=1/|xj_j| per partition
    (|arg| <= beta <= 1, so no row-max needed for stability)
  - adj mask: packed bits unpacked on DVE via (w >> b) & 1 -> {0,1} i16,
    applied to E via an int16 multiply of the f16 bit patterns
    (x*1 = x, x*0 = 0, bitwise-exact)
  - MM2: out[i, 0:257] += Pu.T @ [xj | 1] accumulated over all j in PSUM
  - normalize rows by 1/den on DVE, DMA out as f16
"""
import sys

sys.path.insert(0, "/opt/trn_rl_repo")

import numpy as np

import concourse.bass as bass
import concourse.bacc as bacc
import concourse.tile as tile
from concourse import mybir, masks
from concourse.bass_utils import run_bass_kernel_spmd

F32 = mybir.dt.float32
F16 = mybir.dt.float16
I16 = mybir.dt.int16

N_CORES = 8

# blob layout, in int16 elements (all sections 2-byte)
_XI_LEN = 256 * 1024        # xi_t [256, 1024] f16 (row-major)
_XJS_LEN = 1024 * 256       # xjs [1024, 256] f16
_SJ_LEN = 128 * 64          # sj [128, 64] f16 (partition-major)
_ADJ_LEN = 128 * 4096       # adjp [128, 4096] i16 (partition-major)
_XI_OFF = 0
_XJS_OFF = _XI_OFF + _XI_LEN
_SJ_OFF = _XJS_OFF + _XJS_LEN
_ADJ_OFF = _SJ_OFF + _SJ_LEN
_TOT = _ADJ_OFF + _ADJ_LEN


def build_nc(NI=1024, M=8192, D=256):
    """Per-core program. NI = i-rows per core, M = j-columns, D = features."""
    assert NI == 1024 and M == 8192 and D == 256
    NIB = NI // 128          # i-blocks per core (8)
    NHALF = 2                # i halves
    IBH = NIB // NHALF       # i-blocks per half (4)
    IW = NI // NHALF         # i width per half (512)
    NJB = M // 128           # j-blocks (64)
    NG = NJB // 8            # groups of 8 j-blocks (8)
    DH = D // 128            # d halves (2)
    W = NI // 16             # packed words per j-row (64)
    SH = M // N_CORES        # xj shard rows (1024)

    nc = bacc.Bacc("TRN2", target_bir_lowering=False, debug=False)
    blob = nc.declare_dram_parameter("blob", [_TOT], I16, isOutput=False)
    out = nc.declare_dram_parameter("out", [NI, D], F16, isOutput=True)

    xj_bounce = nc.dram_tensor("xj_bounce", [SH * D], F16, kind="Internal")
    xj_full = nc.dram_tensor("xj_full", [M * D], F16, kind="Internal",
                             addr_space="Shared")

    with tile.TileContext(nc) as tc:
        with (
            tc.tile_pool(name="big", bufs=1) as big,
            tc.tile_pool(name="mpool", bufs=2) as mpool,
            tc.tile_pool(name="epool", bufs=2) as epool,
            tc.tile_pool(name="ppool", bufs=2) as ppool,
            tc.tile_pool(name="outp", bufs=4) as outp,
            tc.tile_pool(name="ps_a", space="PSUM", bufs=2) as ps_a,
            tc.tile_pool(name="ps_o", space="PSUM", bufs=IBH) as ps_o,
        ):
            # ---------------- static tiles ----------------
            ident = big.tile([128, 128], F16)
            xi_sT = big.tile([128, DH, NI], F16)
            sj16 = big.tile([128, NJB], F16)
            sj_sb = big.tile([128, NJB], F32)
            pk = big.tile([128, NJB, W], I16)
            xj_aug = big.tile([128, NJB, D + 1], F16)
            xjT = [big.tile([128, M], F16, name=f"xjT{dh}", tag=f"xjT{dh}")
                   for dh in range(DH)]

            masks.make_identity(nc, ident[:, :])

            # ---------------- prep ----------------
            nc.sync.dma_start(
                out=xj_bounce[:],
                in_=blob[_XJS_OFF:_XJS_OFF + _XJS_LEN].bitcast(F16))
            nc.gpsimd.collective_compute(
                "AllGather", mybir.AluOpType.bypass,
                replica_groups=[list(range(N_CORES))],
                ins=[xj_bounce[:]], outs=[xj_full[:]],
            )
            nc.scalar.dma_start(
                out=xi_sT[:, :, :],
                in_=blob[_XI_OFF:_XI_OFF + _XI_LEN].bitcast(F16)
                .rearrange("(dh p i) -> p dh i", p=128, i=NI))
            nc.scalar.dma_start(
                out=sj16[:, :],
                in_=blob[_SJ_OFF:_SJ_OFF + _SJ_LEN].bitcast(F16)
                .rearrange("(p jb) -> p jb", jb=NJB))
            nc.vector.tensor_copy(sj_sb[:, :], sj16[:, :])
            nc.scalar.dma_start(
                out=pk[:, :, :],
                in_=blob[_ADJ_OFF:_ADJ_OFF + _ADJ_LEN]
                .rearrange("(p jb w) -> p jb w", jb=NJB, w=W))
            nc.vector.memset(xj_aug[:, :, D:D + 1], 1.0)
            nc.scalar.dma_start(
                out=xj_aug[:, :, 0:D],
                in_=xj_full[:].rearrange("(jb p d) -> p jb d", p=128, d=D))

            # PE-transpose xj -> xjT [d, j], 8 j-blocks per PSUM bank (f16)
            for dh in range(DH):
                for g8 in range(NJB // 8):
                    tp = ps_a.tile([128, 8, 128], F16, tag="tp")
                    for q in range(8):
                        jb = 8 * g8 + q
                        nc.tensor.matmul(
                            tp[:, q, :], xj_aug[:, jb, 128 * dh:128 * (dh + 1)],
                            ident[:, :], is_transpose=True)
                    nc.vector.tensor_copy(
                        xjT[dh][:, 1024 * g8:1024 * (g8 + 1)], tp[:, :, :])

            # ---------------- main loop ----------------
            for h in range(NHALF):
                ps_out = [ps_o.tile([128, D + 1], F32, name=f"ps_out_{h}_{b}",
                                    tag="ps_out") for b in range(IBH)]
                for g in range(NG):
                    # unpack 8 j-blocks' mask bits for this i-half -> {0,1} i16
                    mask = mpool.tile([128, 8, IW], I16, tag="mask")
                    for bb in range(8):
                        nc.vector.tensor_scalar(
                            out=mask[:, :, 64 * bb:64 * (bb + 1)],
                            in0=pk[:, 8 * g:8 * (g + 1), :],
                            scalar1=8 * h + bb, scalar2=1,
                            op0=mybir.AluOpType.logical_shift_right,
                            op1=mybir.AluOpType.bitwise_and,
                        )
                    e_ch = epool.tile([128, 8, IW], F16, tag="e")
                    for q in range(8):
                        jb = 8 * g + q
                        st = ps_a.tile([128, IW], F32, tag="st")
                        for dh in range(DH):
                            nc.tensor.matmul(
                                st[:, :],
                                xjT[dh][:, 128 * jb:128 * (jb + 1)],
                                xi_sT[:, dh, IW * h:IW * (h + 1)],
                                start=(dh == 0), stop=(dh == DH - 1),
                            )
                        nc.scalar.activation(
                            out=e_ch[:, q, :], in_=st[:, :],
                            func=mybir.ActivationFunctionType.Exp,
                            scale=sj_sb[:, jb:jb + 1])
                    # Pu = E * mask, as an i16 multiply of the f16 bit patterns
                    pu = ppool.tile([128, 8, IW], F16, tag="pu")
                    nc.vector.tensor_tensor(
                        out=pu[:, :, :].bitcast(I16),
                        in0=e_ch[:, :, :].bitcast(I16),
                        in1=mask[:, :, :], op=mybir.AluOpType.mult)
                    for q in range(8):
                        jb = 8 * g + q
                        for b in range(IBH):
                            nc.tensor.matmul(
                                ps_out[b][:, :],
                                pu[:, q, 128 * b:128 * (b + 1)],
                                xj_aug[:, jb, :],
                                start=(jb == 0), stop=(jb == NJB - 1),
                            )
                # normalize + store
                for b in range(IBH):
                    ib = h * IBH + b
                    rden = outp.tile([128, 1], F32, tag="rden")
                    nc.vector.reciprocal(out=rden[:, :], in_=ps_out[b][:, D:D + 1])
                    of = outp.tile([128, D], F16, tag="of")
                    nc.vector.tensor_scalar(
                        out=of[:, :], in0=ps_out[b][:, 0:D],
                        scalar1=rden[:, 0:1], scalar2=None,
                        op0=mybir.AluOpType.mult)
                    nc.scalar.dma_start(
                        out=out[128 * ib:128 * (ib + 1), :], in_=of[:, :])

    nc.finalize()
    return nc


_NC_CACHE = {}


def _get_nc(NI, M, D):
    key = (NI, M, D)
    if key not in _NC_CACHE:
        _NC_CACHE[key] = build_nc(NI, M, D)
    return _NC_CACHE[key]


def prepare_in_maps(xi, xj, adj, beta):
    """Host-side preprocessing: normalize/fold/pack the raw inputs into one
    compact int16 blob per core."""
    xi = np.asarray(xi, dtype=np.float32)
    xj = np.asarray(xj, dtype=np.float32)
    adj = np.asarray(adj)
    beta = np.asarray(beta, dtype=np.float32)
    N, D = xi.shape
    M = xj.shape[0]
    NI = N // N_CORES
    SH = M // N_CORES
    NJB = M // 128
    W = NI // 16

    b = float(beta.reshape(-1)[0])
    xi_s = (xi * (b / np.linalg.norm(xi, axis=1, keepdims=True))).astype(np.float16)
    xj16 = xj.astype(np.float16)
    sj_all = (1.0 / np.linalg.norm(xj, axis=1)).astype(np.float16)
    # [128, NJB] with [p, jb] = 1/|xj_{128*jb+p}|
    sj_r = np.ascontiguousarray(sj_all.reshape(NJB, 128).T)
    sj_i16 = sj_r.view(np.int16).ravel()

    # pack adj bits: word_c[j, w] bit bb = adj[c*NI + 64*bb + w, j]
    A = (adj != 0).reshape(N_CORES, 16, W, M)
    words = np.zeros((N_CORES, W, M), dtype=np.uint16)
    for bb in range(16):
        words |= A[:, bb, :, :].astype(np.uint16) << bb

    in_maps = []
    for c in range(N_CORES):
        # [128, NJB*W] with [p, jb*W + w] = word_c[128*jb+p, w]
        adjp_c = (
            words[c].T.reshape(NJB, 128, W).transpose(1, 0, 2).reshape(-1)
        ).view(np.int16)
        blob = np.empty(_TOT, dtype=np.int16)
        blob[_XI_OFF:_XI_OFF + _XI_LEN] = \
            np.ascontiguousarray(xi_s[c * NI:(c + 1) * NI].T).view(np.int16).ravel()
        blob[_XJS_OFF:_XJS_OFF + _XJS_LEN] = \
            np.ascontiguousarray(xj16[c * SH:(c + 1) * SH]).view(np.int16).ravel()
        blob[_SJ_OFF:_SJ_OFF + _SJ_LEN] = sj_i16
        blob[_ADJ_OFF:_ADJ_OFF + _ADJ_LEN] = adjp_c
        in_maps.append({"blob": blob})
    return in_maps


def kernel(xi, xj, adj, beta):
    N, D = np.asarray(xi).shape
    M = np.asarray(xj).shape[0]
    NI = N // N_CORES
    nc = _get_nc(NI, M, D)
    in_maps = prepare_in_maps(xi, xj, adj, beta)
    res = run_bass_kernel_spmd(nc, in_maps, list(range(N_CORES)))
    return np.concatenate(
        [res.results[k]["out"] for k in range(N_CORES)], axis=0
    ).astype(np.float32)


# revision 5
# speedup vs baseline: 6.7893x; 1.4675x over previous
"""Trainium2 Bass kernel for CosineGraphAttentionLayer.

reference:
    cos = beta * (xi @ xj.T) / (|xi| |xj| + eps)
    P   = softmax(cos + (1-adj) * -1e9, axis=1)
    out = P @ xj

Sharding: xi/adj row-sharded across 8 cores; xj sharded and AllGathered
on-device (NeuronLink), beta folded into xi host-side.

The dominant cost of this problem under the axon-tunneled harness is
per-execution dispatch overhead, which scales with the NUMBER of input
buffers (~2 ms each) plus shipped bytes (~12 GB/s). So all inputs are
packed host-side into a single compact int16 blob per core:
  - xi pre-normalized (beta/|xi| folded in), transposed, f16
  - xj as f16 shard (AllGathered across the 8 cores on-device)
  - 1/|xj| as f16 (folds into the exp via ACT's per-partition scale)
  - adj bit-packed into int16 words (32x smaller than int32)
and the output is returned as f16, cast to f32 on host.

Per-core kernel (scores in TRANSPOSED [j, i] layout, all matmuls f16):
  - AllGather xj shards -> full xj f16 in DRAM; load as [128, jb, 257]
    with a ones column appended (MM2 rhs; col 256 accumulates the
    softmax denominator)
  - PE-transpose xj blocks -> xjT [d, j] tiles (MM1 lhsT)
  - MM1: ST[j,i] = xjT.T @ xi_sT, f16, N=512
  - exp on ACT straight out of PSUM with scale=1/|xj_j| per partition
    (|arg| <= beta <= 1, so no row-max needed for stability)
  - adj mask: packed bits unpacked on DVE via (w >> b) & 1 -> {0,1} i16,
    applied to E via an int16 multiply of the f16 bit patterns
    (x*1 = x, x*0 = 0, bitwise-exact)
  - MM2: out[i, 0:257] += Pu.T @ [xj | 1] accumulated over all j in PSUM
  - normalize rows by 1/den on DVE, DMA out as f16
"""
import sys

sys.path.insert(0, "/opt/trn_rl_repo")

import numpy as np

import concourse.bass as bass
import concourse.bacc as bacc
import concourse.tile as tile
from concourse import mybir, masks
from concourse.bass_utils import run_bass_kernel_spmd

F32 = mybir.dt.float32
F16 = mybir.dt.float16
I16 = mybir.dt.int16

N_CORES = 8

# blob layout, in int16 elements (all sections 2-byte)
_XI_LEN = 256 * 1024        # xi_t [256, 1024] f16 (row-major)
_XJS_LEN = 1024 * 256       # xjs [1024, 256] f16
_SJ_LEN = 128 * 64          # sj [128, 64] f16 (partition-major)
_ADJ_LEN = 128 * 4096       # adjp [128, 4096] i16 (partition-major)
_XI_OFF = 0
_XJS_OFF = _XI_OFF + _XI_LEN
_SJ_OFF = _XJS_OFF + _XJS_LEN
_ADJ_OFF = _SJ_OFF + _SJ_LEN
_TOT = _ADJ_OFF + _ADJ_LEN


def build_nc(NI=1024, M=8192, D=256):
    """Per-core program. NI = i-rows per core, M = j-columns, D = features."""
    assert NI == 1024 and M == 8192 and D == 256
    NIB = NI // 128          # i-blocks per core (8)
    NHALF = 2                # i halves
    IBH = NIB // NHALF       # i-blocks per half (4)
    IW = NI // NHALF         # i width per half (512)
    NJB = M // 128           # j-blocks (64)
    NG = NJB // 8            # groups of 8 j-blocks (8)
    DH = D // 128            # d halves (2)
    W = NI // 16             # packed words per j-row (64)
    SH = M // N_CORES        # xj shard rows (1024)

    nc = bacc.Bacc("TRN2", target_bir_lowering=False, debug=False,
                   enable_partition_id=False)
    blob = nc.declare_dram_parameter("blob", [_TOT], I16, isOutput=False)
    out = nc.declare_dram_parameter("out", [NI, D], F16, isOutput=True)

    xj_bounce = nc.dram_tensor("xj_bounce", [SH * D], F16, kind="Internal")
    xj_full = nc.dram_tensor("xj_full", [M * D], F16, kind="Internal",
                             addr_space="Shared")

    with tile.TileContext(nc) as tc:
        with (
            tc.tile_pool(name="big", bufs=1) as big,
            tc.tile_pool(name="mpool", bufs=2) as mpool,
            tc.tile_pool(name="epool", bufs=2) as epool,
            tc.tile_pool(name="ppool", bufs=2) as ppool,
            tc.tile_pool(name="outp", bufs=4) as outp,
            tc.tile_pool(name="ps_a", space="PSUM", bufs=2) as ps_a,
            tc.tile_pool(name="ps_o", space="PSUM", bufs=IBH) as ps_o,
        ):
            # ---------------- static tiles ----------------
            ident = big.tile([128, 128], F16)
            xi_sT = big.tile([128, DH, NI], F16)
            sj16 = big.tile([128, NJB], F16)
            sj_sb = big.tile([128, NJB], F32)
            pk = big.tile([128, NJB, W], I16)
            xj_aug = big.tile([128, NJB, D + 1], F16)
            xjT = [big.tile([128, M], F16, name=f"xjT{dh}", tag=f"xjT{dh}")
                   for dh in range(DH)]

            masks.make_identity(nc, ident[:, :])

            # ---------------- prep ----------------
            nc.sync.dma_start(
                out=xj_bounce[:],
                in_=blob[_XJS_OFF:_XJS_OFF + _XJS_LEN].bitcast(F16))
            nc.gpsimd.collective_compute(
                "AllGather", mybir.AluOpType.bypass,
                replica_groups=[list(range(N_CORES))],
                ins=[xj_bounce[:]], outs=[xj_full[:]],
            )
            nc.scalar.dma_start(
                out=xi_sT[:, :, :],
                in_=blob[_XI_OFF:_XI_OFF + _XI_LEN].bitcast(F16)
                .rearrange("(dh p i) -> p dh i", p=128, i=NI))
            nc.scalar.dma_start(
                out=sj16[:, :],
                in_=blob[_SJ_OFF:_SJ_OFF + _SJ_LEN].bitcast(F16)
                .rearrange("(p jb) -> p jb", jb=NJB))
            nc.vector.tensor_copy(sj_sb[:, :], sj16[:, :])
            nc.scalar.dma_start(
                out=pk[:, :, :],
                in_=blob[_ADJ_OFF:_ADJ_OFF + _ADJ_LEN]
                .rearrange("(p jb w) -> p jb w", jb=NJB, w=W))
            nc.vector.memset(xj_aug[:, :, D:D + 1], 1.0)
            nc.scalar.dma_start(
                out=xj_aug[:, :, 0:D],
                in_=xj_full[:].rearrange("(jb p d) -> p jb d", p=128, d=D))

            # PE-transpose xj -> xjT [d, j], 8 j-blocks per PSUM bank (f16)
            for dh in range(DH):
                for g8 in range(NJB // 8):
                    tp = ps_a.tile([128, 8, 128], F16, tag="tp")
                    for q in range(8):
                        jb = 8 * g8 + q
                        nc.tensor.matmul(
                            tp[:, q, :], xj_aug[:, jb, 128 * dh:128 * (dh + 1)],
                            ident[:, :], is_transpose=True)
                    nc.vector.tensor_copy(
                        xjT[dh][:, 1024 * g8:1024 * (g8 + 1)], tp[:, :, :])

            # ---------------- main loop ----------------
            for h in range(NHALF):
                ps_out = [ps_o.tile([128, D + 1], F32, name=f"ps_out_{h}_{b}",
                                    tag="ps_out") for b in range(IBH)]
                for g in range(NG):
                    # unpack 8 j-blocks' mask bits for this i-half -> {0,1} i16
                    mask = mpool.tile([128, 8, IW], I16, tag="mask")
                    for bb in range(8):
                        nc.vector.tensor_scalar(
                            out=mask[:, :, 64 * bb:64 * (bb + 1)],
                            in0=pk[:, 8 * g:8 * (g + 1), :],
                            scalar1=8 * h + bb, scalar2=1,
                            op0=mybir.AluOpType.logical_shift_right,
                            op1=mybir.AluOpType.bitwise_and,
                        )
                    e_ch = epool.tile([128, 8, IW], F16, tag="e")
                    for q in range(8):
                        jb = 8 * g + q
                        st = ps_a.tile([128, IW], F32, tag="st")
                        for dh in range(DH):
                            nc.tensor.matmul(
                                st[:, :],
                                xjT[dh][:, 128 * jb:128 * (jb + 1)],
                                xi_sT[:, dh, IW * h:IW * (h + 1)],
                                start=(dh == 0), stop=(dh == DH - 1),
                            )
                        nc.scalar.activation(
                            out=e_ch[:, q, :], in_=st[:, :],
                            func=mybir.ActivationFunctionType.Exp,
                            scale=sj_sb[:, jb:jb + 1])
                    # Pu = E * mask, as an i16 multiply of the f16 bit patterns
                    pu = ppool.tile([128, 8, IW], F16, tag="pu")
                    nc.vector.tensor_tensor(
                        out=pu[:, :, :].bitcast(I16),
                        in0=e_ch[:, :, :].bitcast(I16),
                        in1=mask[:, :, :], op=mybir.AluOpType.mult)
                    for q in range(8):
                        jb = 8 * g + q
                        for b in range(IBH):
                            nc.tensor.matmul(
                                ps_out[b][:, :],
                                pu[:, q, 128 * b:128 * (b + 1)],
                                xj_aug[:, jb, :],
                                start=(jb == 0), stop=(jb == NJB - 1),
                            )
                # normalize + store
                for b in range(IBH):
                    ib = h * IBH + b
                    rden = outp.tile([128, 1], F32, tag="rden")
                    nc.vector.reciprocal(out=rden[:, :], in_=ps_out[b][:, D:D + 1])
                    of = outp.tile([128, D], F16, tag="of")
                    nc.vector.tensor_scalar(
                        out=of[:, :], in0=ps_out[b][:, 0:D],
                        scalar1=rden[:, 0:1], scalar2=None,
                        op0=mybir.AluOpType.mult)
                    nc.scalar.dma_start(
                        out=out[128 * ib:128 * (ib + 1), :], in_=of[:, :])

    nc.finalize()
    return nc


_NC_CACHE = {}


def _get_nc(NI, M, D):
    key = (NI, M, D)
    if key not in _NC_CACHE:
        _NC_CACHE[key] = build_nc(NI, M, D)
    return _NC_CACHE[key]


def prepare_in_maps(xi, xj, adj, beta):
    """Host-side preprocessing: normalize/fold/pack the raw inputs into one
    compact int16 blob per core."""
    xi = np.asarray(xi, dtype=np.float32)
    xj = np.asarray(xj, dtype=np.float32)
    adj = np.asarray(adj)
    beta = np.asarray(beta, dtype=np.float32)
    N, D = xi.shape
    M = xj.shape[0]
    NI = N // N_CORES
    SH = M // N_CORES
    NJB = M // 128
    W = NI // 16

    b = float(beta.reshape(-1)[0])
    xi_s = (xi * (b / np.linalg.norm(xi, axis=1, keepdims=True))).astype(np.float16)
    xj16 = xj.astype(np.float16)
    sj_all = (1.0 / np.linalg.norm(xj, axis=1)).astype(np.float16)
    # [128, NJB] with [p, jb] = 1/|xj_{128*jb+p}|
    sj_r = np.ascontiguousarray(sj_all.reshape(NJB, 128).T)
    sj_i16 = sj_r.view(np.int16).ravel()

    # pack adj bits: word_c[j, w] bit bb = adj[c*NI + 64*bb + w, j]
    A = (adj != 0).reshape(N_CORES, 16, W, M)
    words = np.zeros((N_CORES, W, M), dtype=np.uint16)
    for bb in range(16):
        words |= A[:, bb, :, :].astype(np.uint16) << bb

    in_maps = []
    for c in range(N_CORES):
        # [128, NJB*W] with [p, jb*W + w] = word_c[128*jb+p, w]
        adjp_c = (
            words[c].T.reshape(NJB, 128, W).transpose(1, 0, 2).reshape(-1)
        ).view(np.int16)
        blob = np.empty(_TOT, dtype=np.int16)
        blob[_XI_OFF:_XI_OFF + _XI_LEN] = \
            np.ascontiguousarray(xi_s[c * NI:(c + 1) * NI].T).view(np.int16).ravel()
        blob[_XJS_OFF:_XJS_OFF + _XJS_LEN] = \
            np.ascontiguousarray(xj16[c * SH:(c + 1) * SH]).view(np.int16).ravel()
        blob[_SJ_OFF:_SJ_OFF + _SJ_LEN] = sj_i16
        blob[_ADJ_OFF:_ADJ_OFF + _ADJ_LEN] = adjp_c
        in_maps.append({"blob": blob})
    return in_maps


def kernel(xi, xj, adj, beta):
    N, D = np.asarray(xi).shape
    M = np.asarray(xj).shape[0]
    NI = N // N_CORES
    nc = _get_nc(NI, M, D)
    in_maps = prepare_in_maps(xi, xj, adj, beta)
    res = run_bass_kernel_spmd(nc, in_maps, list(range(N_CORES)))
    return np.concatenate(
        [res.results[k]["out"] for k in range(N_CORES)], axis=0
    ).astype(np.float32)


# revision 8
# speedup vs baseline: 12.5973x; 1.8555x over previous
"""Trainium2 Bass kernel for CosineGraphAttentionLayer.

reference:
    cos = beta * (xi @ xj.T) / (|xi| |xj| + eps)
    P   = softmax(cos + (1-adj) * -1e9, axis=1)
    out = P @ xj

Sharding: xi/adj row-sharded across N_CORES cores; xj sharded and
AllGathered on-device (NeuronLink), beta folded into xi host-side.

The dominant cost of this problem under the axon-tunneled harness is
per-execution dispatch overhead, which scales with the NUMBER of input
buffers (~1-2 ms each) and participating cores, plus shipped bytes
(~12 GB/s). So all inputs are packed host-side into a single compact
int16 blob per core:
  - xi pre-normalized (beta/|xi| folded in), transposed, f16
  - xj as f16 shard (AllGathered across cores on-device; shard rows
    interleaved so each chunked sub-AllGather yields a contiguous
    j-range, letting compute start before the full gather lands)
  - 1/|xj| as f16 (folds into the exp via ACT's per-partition scale)
  - adj bit-packed into int16 words (32x smaller than int32)
and the output is returned as f16, cast to f32 on host.

Per-core kernel (scores in TRANSPOSED [j, i] layout, all matmuls f16):
  - chunked AllGather xj shards -> full xj f16 in DRAM; load as
    [128, jb, 257] with a ones column appended (MM2 rhs; col 256
    accumulates the softmax denominator)
  - PE-transpose xj blocks -> xjT [d, j] tiles (MM1 lhsT)
  - MM1: ST[j,i] = xjT.T @ xi_sT, f16, N=512
  - exp on ACT straight out of PSUM with scale=1/|xj_j| per partition
    (|arg| <= beta <= 1, so no row-max needed for stability)
  - adj mask: packed bits unpacked on DVE via (w >> b) & 1 -> {0,1} i16,
    applied to E via an int16 multiply of the f16 bit patterns
    (x*1 = x, x*0 = 0, bitwise-exact)
  - MM2: out[i, 0:257] += Pu.T @ [xj | 1] accumulated over all j in PSUM
  - normalize rows by 1/den on DVE, DMA out as f16
"""
import sys

sys.path.insert(0, "/opt/trn_rl_repo")

import numpy as np

import concourse.bass as bass
import concourse.bacc as bacc
import concourse.tile as tile
from concourse import mybir, masks
from concourse.bass_utils import run_bass_kernel_spmd

F32 = mybir.dt.float32
F16 = mybir.dt.float16
I16 = mybir.dt.int16

N_CORES = 2
_N, _M, _D = 8192, 8192, 256
_CH = 4                      # AllGather chunks

_NI = _N // N_CORES          # i-rows per core
# xj "shard": below 4 cores the AllGather path is unsupported/not worth it --
# ship the full xj in each core's blob instead and skip the collective.
_USE_AG = N_CORES > 2
_SH = (_M // N_CORES) if _USE_AG else _M
_W = _NI // 16               # packed words per j-row

# blob layout, in int16 elements (all sections 2-byte)
_XI_LEN = _D * _NI           # xi_t [D, NI] f16 (row-major)
_XJS_LEN = _SH * _D          # xjs [SH, D] f16 (chunk-interleaved rows)
_SJ_LEN = 128 * (_M // 128)  # sj [128, NJB] f16 (partition-major)
_ADJ_LEN = 128 * (_M // 128) * _W   # adjp [128, NJB*W] i16 (partition-major)
_XI_OFF = 0
_XJS_OFF = _XI_OFF + _XI_LEN
_SJ_OFF = _XJS_OFF + _XJS_LEN
_ADJ_OFF = _SJ_OFF + _SJ_LEN
_TOT = _ADJ_OFF + _ADJ_LEN


def build_nc(NI=_NI, M=_M, D=_D):
    """Per-core program. NI = i-rows per core, M = j-columns, D = features."""
    assert NI == _NI and M == _M and D == _D
    NIB = NI // 128          # i-blocks per core
    NHALF = NI // 512        # i halves (512-wide score strips)
    IBH = 4                  # i-blocks per half
    IW = 512                 # i width per half
    NJB = M // 128           # j-blocks (64)
    NG = NJB // 8            # groups of 8 j-blocks (8)
    DH = D // 128            # d halves (2)
    W = _W                   # packed words per j-row
    BPH = IW // W            # packed bits per i-half
    SH = _SH                 # xj shard rows
    CSH = SH // _CH          # shard rows contributed per AG chunk
    MCH = M // _CH           # gathered rows per AG chunk
    JBCH = MCH // 128        # j-blocks per AG chunk

    nc = bacc.Bacc("TRN2", target_bir_lowering=False, debug=False,
                   enable_partition_id=False)
    blob = nc.declare_dram_parameter("blob", [_TOT], I16, isOutput=False)
    out = nc.declare_dram_parameter("out", [NI, D], F16, isOutput=True)

    if _USE_AG:
        xj_bounce = nc.dram_tensor("xj_bounce", [SH * D], F16, kind="Internal")
        # Shared scratchpad outputs are only supported for >4-core groups
        xj_full = nc.dram_tensor(
            "xj_full", [M * D], F16, kind="Internal",
            addr_space="Shared" if N_CORES > 4 else "Local")
    else:
        xj_full = None

    with tile.TileContext(nc) as tc:
        with (
            tc.tile_pool(name="big", bufs=1) as big,
            tc.tile_pool(name="mpool", bufs=2) as mpool,
            tc.tile_pool(name="epool", bufs=2) as epool,
            tc.tile_pool(name="ppool", bufs=2) as ppool,
            tc.tile_pool(name="outp", bufs=4) as outp,
            tc.tile_pool(name="ps_a", space="PSUM", bufs=2) as ps_a,
            tc.tile_pool(name="ps_o", space="PSUM", bufs=IBH) as ps_o,
        ):
            # ---------------- static tiles ----------------
            ident = big.tile([128, 128], F16)
            xi_sT = big.tile([128, DH, NI], F16)
            sj16 = big.tile([128, NJB], F16)
            sj_sb = big.tile([128, NJB], F32)
            pk = big.tile([128, NJB, W], I16)
            xj_aug = big.tile([128, NJB, D + 1], F16)
            xjT = [big.tile([128, M], F16, name=f"xjT{dh}", tag=f"xjT{dh}")
                   for dh in range(DH)]

            masks.make_identity(nc, ident[:, :])

            # ---------------- prep ----------------
            if _USE_AG:
                nc.sync.dma_start(
                    out=xj_bounce[:],
                    in_=blob[_XJS_OFF:_XJS_OFF + _XJS_LEN].bitcast(F16))
                for s in range(_CH):
                    nc.gpsimd.collective_compute(
                        "AllGather", mybir.AluOpType.bypass,
                        replica_groups=[list(range(N_CORES))],
                        ins=[xj_bounce[CSH * D * s:CSH * D * (s + 1)]],
                        outs=[xj_full[MCH * D * s:MCH * D * (s + 1)]],
                    )
            nc.scalar.dma_start(
                out=xi_sT[:, :, :],
                in_=blob[_XI_OFF:_XI_OFF + _XI_LEN].bitcast(F16)
                .rearrange("(dh p i) -> p dh i", p=128, i=NI))
            nc.scalar.dma_start(
                out=sj16[:, :],
                in_=blob[_SJ_OFF:_SJ_OFF + _SJ_LEN].bitcast(F16)
                .rearrange("(p jb) -> p jb", jb=NJB))
            nc.vector.tensor_copy(sj_sb[:, :], sj16[:, :])
            nc.scalar.dma_start(
                out=pk[:, :, :],
                in_=blob[_ADJ_OFF:_ADJ_OFF + _ADJ_LEN]
                .rearrange("(p jb w) -> p jb w", jb=NJB, w=W))
            nc.vector.memset(xj_aug[:, :, D:D + 1], 1.0)
            for s in range(_CH):
                if _USE_AG:
                    src = xj_full[MCH * D * s:MCH * D * (s + 1)]
                else:
                    src = blob[_XJS_OFF + MCH * D * s:
                               _XJS_OFF + MCH * D * (s + 1)].bitcast(F16)
                nc.scalar.dma_start(
                    out=xj_aug[:, JBCH * s:JBCH * (s + 1), 0:D],
                    in_=src.rearrange("(jb p d) -> p jb d", p=128, d=D))

            # PE-transpose xj -> xjT [d, j], 8 j-blocks per PSUM bank (f16)
            for g8 in range(NJB // 8):
                for dh in range(DH):
                    tp = ps_a.tile([128, 8, 128], F16, tag="tp")
                    for q in range(8):
                        jb = 8 * g8 + q
                        nc.tensor.matmul(
                            tp[:, q, :], xj_aug[:, jb, 128 * dh:128 * (dh + 1)],
                            ident[:, :], is_transpose=True)
                    nc.vector.tensor_copy(
                        xjT[dh][:, 1024 * g8:1024 * (g8 + 1)], tp[:, :, :])

            # ---------------- main loop ----------------
            for h in range(NHALF):
                ps_out = [ps_o.tile([128, D + 1], F32, name=f"ps_out_{h}_{b}",
                                    tag="ps_out") for b in range(IBH)]
                for g in range(NG):
                    # unpack 8 j-blocks' mask bits for this i-half -> {0,1} i16
                    mask = mpool.tile([128, 8, IW], I16, tag="mask")
                    for t in range(BPH):
                        nc.vector.tensor_scalar(
                            out=mask[:, :, W * t:W * (t + 1)],
                            in0=pk[:, 8 * g:8 * (g + 1), :],
                            scalar1=BPH * h + t, scalar2=1,
                            op0=mybir.AluOpType.logical_shift_right,
                            op1=mybir.AluOpType.bitwise_and,
                        )
                    e_ch = epool.tile([128, 8, IW], F16, tag="e")
                    for q in range(8):
                        jb = 8 * g + q
                        st = ps_a.tile([128, IW], F32, tag="st")
                        for dh in range(DH):
                            nc.tensor.matmul(
                                st[:, :],
                                xjT[dh][:, 128 * jb:128 * (jb + 1)],
                                xi_sT[:, dh, IW * h:IW * (h + 1)],
                                start=(dh == 0), stop=(dh == DH - 1),
                            )
                        nc.scalar.activation(
                            out=e_ch[:, q, :], in_=st[:, :],
                            func=mybir.ActivationFunctionType.Exp,
                            scale=sj_sb[:, jb:jb + 1])
                    # Pu = E * mask, as an i16 multiply of the f16 bit patterns
                    pu = ppool.tile([128, 8, IW], F16, tag="pu")
                    nc.vector.tensor_tensor(
                        out=pu[:, :, :].bitcast(I16),
                        in0=e_ch[:, :, :].bitcast(I16),
                        in1=mask[:, :, :], op=mybir.AluOpType.mult)
                    for q in range(8):
                        jb = 8 * g + q
                        for b in range(IBH):
                            nc.tensor.matmul(
                                ps_out[b][:, :],
                                pu[:, q, 128 * b:128 * (b + 1)],
                                xj_aug[:, jb, :],
                                start=(jb == 0), stop=(jb == NJB - 1),
                            )
                # normalize + store
                for b in range(IBH):
                    ib = h * IBH + b
                    rden = outp.tile([128, 1], F32, tag="rden")
                    nc.vector.reciprocal(out=rden[:, :], in_=ps_out[b][:, D:D + 1])
                    of = outp.tile([128, D], F16, tag="of")
                    nc.vector.tensor_scalar(
                        out=of[:, :], in0=ps_out[b][:, 0:D],
                        scalar1=rden[:, 0:1], scalar2=None,
                        op0=mybir.AluOpType.mult)
                    nc.scalar.dma_start(
                        out=out[128 * ib:128 * (ib + 1), :], in_=of[:, :])

    nc.finalize()
    return nc


_NC_CACHE = {}


def _get_nc(NI, M, D):
    key = (NI, M, D)
    if key not in _NC_CACHE:
        _NC_CACHE[key] = build_nc(NI, M, D)
    return _NC_CACHE[key]


def _shard_rows(c):
    """Chunk-interleaved xj shard row indices for core c (so AG chunk s
    gathers the contiguous j-range [M/CH*s, M/CH*(s+1)) across cores)."""
    CSH = _SH // _CH
    return np.concatenate([
        np.arange(_M // _CH * s + CSH * c, _M // _CH * s + CSH * (c + 1))
        for s in range(_CH)
    ])


def prepare_in_maps(xi, xj, adj, beta):
    """Host-side preprocessing: normalize/fold/pack the raw inputs into one
    compact int16 blob per core."""
    xi = np.asarray(xi, dtype=np.float32)
    xj = np.asarray(xj, dtype=np.float32)
    adj = np.asarray(adj)
    beta = np.asarray(beta, dtype=np.float32)
    N, D = xi.shape
    M = xj.shape[0]
    NI = N // N_CORES
    NJB = M // 128
    W = _W

    b = float(beta.reshape(-1)[0])
    xi_s = (xi * (b / np.linalg.norm(xi, axis=1, keepdims=True))).astype(np.float16)
    xj16 = xj.astype(np.float16)
    sj_all = (1.0 / np.linalg.norm(xj, axis=1)).astype(np.float16)
    # [128, NJB] with [p, jb] = 1/|xj_{128*jb+p}|
    sj_r = np.ascontiguousarray(sj_all.reshape(NJB, 128).T)
    sj_i16 = sj_r.view(np.int16).ravel()

    # pack adj bits: word_c[j, w] bit bb = adj[c*NI + W*bb + w, j]
    A = (adj != 0).reshape(N_CORES, 16, W, M)
    words = np.zeros((N_CORES, W, M), dtype=np.uint16)
    for bb in range(16):
        words |= A[:, bb, :, :].astype(np.uint16) << bb

    in_maps = []
    for c in range(N_CORES):
        # [128, NJB*W] with [p, jb*W + w] = word_c[128*jb+p, w]
        adjp_c = (
            words[c].T.reshape(NJB, 128, W).transpose(1, 0, 2).reshape(-1)
        ).view(np.int16)
        blob = np.empty(_TOT, dtype=np.int16)
        blob[_XI_OFF:_XI_OFF + _XI_LEN] = \
            np.ascontiguousarray(xi_s[c * NI:(c + 1) * NI].T).view(np.int16).ravel()
        blob[_XJS_OFF:_XJS_OFF + _XJS_LEN] = \
            (np.ascontiguousarray(xj16[_shard_rows(c)]) if _USE_AG
             else xj16).view(np.int16).ravel()
        blob[_SJ_OFF:_SJ_OFF + _SJ_LEN] = sj_i16
        blob[_ADJ_OFF:_ADJ_OFF + _ADJ_LEN] = adjp_c
        in_maps.append({"blob": blob})
    return in_maps


def kernel(xi, xj, adj, beta):
    N, D = np.asarray(xi).shape
    M = np.asarray(xj).shape[0]
    NI = N // N_CORES
    nc = _get_nc(NI, M, D)
    in_maps = prepare_in_maps(xi, xj, adj, beta)
    res = run_bass_kernel_spmd(nc, in_maps, list(range(N_CORES)))
    return np.concatenate(
        [res.results[k]["out"] for k in range(N_CORES)], axis=0
    ).astype(np.float32)
